# revision 4
# baseline (speedup 1.0000x reference)
"""Trainium2 Bass kernel for nn_HGNNEncoder (gnn_message_passing).

8-core SPMD: bonds and atoms sharded contiguously across cores; the f16
message / atom-message tables are AllGather-replicated each hop so the
random-index gathers stay core-local (HBM gathers via indirect DMA).

Self-contained: hardcodes the problem shapes from spec.json.
"""
import numpy as np

import concourse.bass as bass
import concourse.mybir as mybir
import concourse.tile as tile
from concourse import bacc
from concourse.bass import IndirectOffsetOnAxis
from concourse.bass_utils import run_bass_kernel_spmd
from concourse.masks import make_identity

P = 128
H = 128
NB = 6
DEPTH = 4
NCORES = 8
SEGR = 32768            # dma_gather int16 segment rows
CAP = 2560              # padded requests per (jg, seg) cell
CAPCOL = CAP // 16

F32 = mybir.dt.float32
F16 = mybir.dt.float16
I32 = mybir.dt.int32
I16 = mybir.dt.int16


def build_nc(A, B, AF, S):
    """Build the SPMD Bass program (identical on all cores)."""
    As = A // NCORES            # atoms per core
    Bs = B // NCORES            # bonds per core
    nblkA = As // P             # atom blocks
    nblkB = Bs // P             # bond blocks
    Ms = As // S                # molecules per core
    MPB = P // S                # molecules per 128-atom block

    nc = bacc.Bacc("TRN2", target_bir_lowering=False, num_devices=NCORES)

    # ---------------- I/O ----------------
    fb = nc.dram_tensor("fb", [Bs, 147], F32, kind="ExternalInput")
    fa = nc.dram_tensor("fa", [As, 134], F32, kind="ExternalInput")  # f_atoms + ones col
    idxA = nc.dram_tensor("idxA", [P, nblkA * NB], I32, kind="ExternalInput")
    idxR = nc.dram_tensor("idxR", [P, nblkB], I32, kind="ExternalInput")
    idxB = nc.dram_tensor("idxB", [P, nblkB], I32, kind="ExternalInput")
    SEGR_v = min(SEGR, B)
    NSEG = B // SEGR_v
    CAP_v = min(CAP, As)
    CAPCOL_v = CAP_v // 16
    ixg = nc.dram_tensor("ixg", [P, NB * NSEG * CAPCOL_v], I16, kind="ExternalInput")
    ixs = nc.dram_tensor("ixs", [P, NB * NSEG * CAPCOL_v], I16, kind="ExternalInput")
    am6 = nc.dram_tensor("am6", [NB * As, H], F16, kind="Internal")
    w_i = nc.dram_tensor("w_i", [147, H], F32, kind="ExternalInput")
    w_h = nc.dram_tensor("w_h", [H, H], F16, kind="ExternalInput")
    w_o = nc.dram_tensor("w_o", [262, H], F32, kind="ExternalInput")  # b_o folded at row 133
    w_a = nc.dram_tensor("w_a", [H, H], F32, kind="ExternalInput")
    w_b = nc.dram_tensor("w_b", [H, H], F32, kind="ExternalInput")
    amask = nc.dram_tensor("amask", [P, P], F32, kind="ExternalInput")  # additive softmax mask
    gsel = nc.dram_tensor("gsel", [P, MPB], F32, kind="ExternalInput")  # mol selector / S

    mv = nc.dram_tensor("mv", [Ms, H], F32, kind="ExternalOutput")

    # ---------------- internals ----------------
    inputs_d = nc.dram_tensor("inputs_d", [Bs, H], F32, kind="Internal")
    m_sh = [nc.dram_tensor(f"m_sh{i}", [Bs, H], F16, kind="Internal") for i in range(2)]
    am_sh = nc.dram_tensor("am_sh", [As, H], F16, kind="Internal")
    m_full = [nc.dram_tensor(f"m_full{i}", [B, H], F16, kind="Internal",
                             addr_space="Shared") for i in range(2)]
    am_full = nc.dram_tensor("am_full", [A, H], F16, kind="Internal",
                             addr_space="Shared")

    RG = [list(range(NCORES))]

    with tile.TileContext(nc) as tc:
        with tc.tile_pool(name="const", bufs=1) as cp, \
             tc.tile_pool(name="gath", bufs=16) as gp, \
             tc.tile_pool(name="gtok", bufs=2) as gq, \
             tc.tile_pool(name="ixq", bufs=2) as xq, \
             tc.tile_pool(name="comb", bufs=1) as cq, \
             tc.tile_pool(name="work", bufs=6) as wp, \
             tc.tile_pool(name="stage", bufs=3) as sp, \
             tc.tile_pool(name="psum", bufs=2, space="PSUM") as pp, \
             tc.tile_pool(name="psum2", bufs=2, space="PSUM") as pp2:

            # constants
            id32 = cp.tile([P, P], F32)
            make_identity(nc, id32[:])
            id16 = cp.tile([P, P], F16)
            nc.vector.tensor_copy(id16[:], id32[:])
            wi_t = cp.tile([P, H], F32, tag="wi1")
            nc.sync.dma_start(out=wi_t[:], in_=w_i[0:128, :])
            wi2_t = cp.tile([P, H], F32, tag="wi2")
            nc.sync.dma_start(out=wi2_t[:19, :], in_=w_i[128:147, :])
            wh_t = cp.tile([P, H], F16, tag="wh")
            nc.sync.dma_start(out=wh_t[:], in_=w_h[:])
            wo1_t = cp.tile([P, H], F32, tag="wo1")
            nc.sync.dma_start(out=wo1_t[:], in_=w_o[0:128, :])
            wo2_t = cp.tile([P, H], F32, tag="wo2")
            nc.sync.dma_start(out=wo2_t[:6, :], in_=w_o[128:134, :])
            wo3_t = cp.tile([P, H], F32, tag="wo3")
            nc.sync.dma_start(out=wo3_t[:], in_=w_o[134:262, :])
            wa_t = cp.tile([P, H], F32, tag="wa")
            nc.sync.dma_start(out=wa_t[:], in_=w_a[:])
            wb_t = cp.tile([P, H], F32, tag="wb")
            nc.sync.dma_start(out=wb_t[:], in_=w_b[:])
            mask_t = cp.tile([P, P], F32, tag="mask")
            nc.sync.dma_start(out=mask_t[:], in_=amask[:])
            g_t = cp.tile([P, MPB], F32, tag="gsel")
            nc.sync.dma_start(out=g_t[:], in_=gsel[:])
            ixA_t = cp.tile([P, nblkA * NB], I32, tag="ixA")
            nc.sync.dma_start(out=ixA_t[:], in_=idxA[:])
            ixR_t = cp.tile([P, nblkB], I32, tag="ixR")
            nc.sync.dma_start(out=ixR_t[:], in_=idxR[:])
            ixB_t = cp.tile([P, nblkB], I32, tag="ixB")
            nc.sync.dma_start(out=ixB_t[:], in_=idxB[:])
            zt = cp.tile([P, 8192], F16, tag="zt")
            nc.vector.memset(zt[:], 0.0)

            def atom_phase_v3(mf):
                """am_sh[a] = sum_j mf[a2b[a, j]] via segmented dma_gather +
                per-jg dma_scatter_add accumulators, then 6-way combine."""
                zrows = min(8192 * P // H, NB * As)
                nzc = (NB * As + zrows - 1) // zrows
                zcols = zrows * H // P
                for r in range(nzc):
                    nc.sync.dma_start(out=am6[r * zrows:(r + 1) * zrows, :],
                                      in_=zt[:, 0:zcols])
                for jg in range(NB):
                    ixg_t = xq.tile([P, NSEG * CAPCOL_v], I16, tag="ixg")
                    nc.sync.dma_start(
                        out=ixg_t[:],
                        in_=ixg[:, jg * NSEG * CAPCOL_v:(jg + 1) * NSEG * CAPCOL_v])
                    ixs_t = xq.tile([P, NSEG * CAPCOL_v], I16, tag="ixs")
                    nc.sync.dma_start(
                        out=ixs_t[:],
                        in_=ixs[:, jg * NSEG * CAPCOL_v:(jg + 1) * NSEG * CAPCOL_v])
                    for seg in range(NSEG):
                        g = gq.tile([P, (CAP_v // P) * H], F16, tag="gtok")
                        nc.gpsimd.dma_gather(
                            out_ap=g[:].rearrange("p (c h) -> p c h", h=H),
                            in_ap=mf[seg * SEGR_v:(seg + 1) * SEGR_v, :],
                            idxs_ap=ixg_t[:, seg * CAPCOL_v:(seg + 1) * CAPCOL_v],
                            num_idxs=CAP_v, num_idxs_reg=CAP_v, elem_size=H,
                            single_packet=False)
                        nc.gpsimd.dma_scatter_add(
                            out_ap=am6[jg * As:(jg + 1) * As, :],
                            in_ap=g[:].rearrange("p (c h) -> p c h", h=H),
                            idxs_ap=ixs_t[:, seg * CAPCOL_v:(seg + 1) * CAPCOL_v],
                            num_idxs=CAP_v, num_idxs_reg=CAP_v, elem_size=H,
                            single_packet=False)
                GKC = 8
                for g0 in range(0, nblkA, GKC):
                    lts = []
                    for jg in range(NB):
                        lt = cq.tile([P, GKC * H], F16, tag=f"lt{jg}")
                        nc.sync.dma_start(
                            out=lt[:].rearrange("p (k h) -> p k h", h=H),
                            in_=am6[jg * As + g0 * P:jg * As + (g0 + GKC) * P, :]
                            .rearrange("(k b) h -> b k h", b=P))
                        lts.append(lt)
                    s01 = cq.tile([P, GKC * H], F16, tag="s01")
                    nc.vector.tensor_add(s01[:], lts[0][:], lts[1][:])
                    s23 = cq.tile([P, GKC * H], F16, tag="s23")
                    nc.vector.tensor_add(s23[:], lts[2][:], lts[3][:])
                    s45 = cq.tile([P, GKC * H], F16, tag="s45")
                    nc.vector.tensor_add(s45[:], lts[4][:], lts[5][:])
                    s03 = cq.tile([P, GKC * H], F16, tag="s03")
                    nc.vector.tensor_add(s03[:], s01[:], s23[:])
                    am16g = cq.tile([P, GKC * H], F16, tag="am16g")
                    nc.vector.tensor_add(am16g[:], s03[:], s45[:])
                    nc.sync.dma_start(
                        out=am_sh[g0 * P:(g0 + GKC) * P, :]
                        .rearrange("(k b) h -> b k h", b=P),
                        in_=am16g[:].rearrange("p (k h) -> p k h", h=H))
                # fixup: pads scatter-added junk onto atom 0; recompute block 0
                gs0 = []
                for j in range(NB):
                    gx = gp.tile([P, H], F16, tag=f"g{j}")
                    nc.gpsimd.indirect_dma_start(
                        out=gx[:], out_offset=None, in_=mf[:],
                        in_offset=IndirectOffsetOnAxis(
                            ap=ixA_t[:, j:j + 1], axis=0))
                    gs0.append(gx)
                f01 = cq.tile([P, H], F32, tag="f01")
                nc.vector.tensor_add(f01[:], gs0[0][:], gs0[1][:])
                f23 = cq.tile([P, H], F32, tag="f23")
                nc.vector.tensor_add(f23[:], gs0[2][:], gs0[3][:])
                f45 = cq.tile([P, H], F32, tag="f45")
                nc.vector.tensor_add(f45[:], gs0[4][:], gs0[5][:])
                f03 = cq.tile([P, H], F32, tag="f03")
                nc.vector.tensor_add(f03[:], f01[:], f23[:])
                fam = cq.tile([P, H], F16, tag="fam")
                nc.vector.tensor_add(fam[:], f03[:], f45[:])
                nc.sync.dma_start(out=am_sh[0:P, :], in_=fam[:])

            # ---------------- phase 0: inputs = fb @ W_i; m0 = relu ----------------
            for blk in range(nblkB):
                r0, r1 = blk * P, (blk + 1) * P
                fb_t = wp.tile([P, 147], F32, tag="fb")
                nc.sync.dma_start(out=fb_t[:], in_=fb[r0:r1, :])
                pt1 = pp.tile([P, P], F32, tag="tp")
                nc.tensor.transpose(pt1[:], fb_t[:, 0:128], id32[:])
                t1 = wp.tile([P, P], F32, tag="t1")
                nc.vector.tensor_copy(t1[:], pt1[:])
                pt2 = pp.tile([P, P], F32, tag="tp")
                nc.tensor.transpose(pt2[:19, :], fb_t[:, 128:147], id32[:])
                t2 = wp.tile([P, P], F32, tag="t2")
                nc.vector.tensor_copy(t2[:19, :], pt2[:19, :])
                pm = pp2.tile([P, P], F32, tag="mm")
                nc.tensor.matmul(pm[:], lhsT=t1[:], rhs=wi_t[:], start=True, stop=False)
                nc.tensor.matmul(pm[:], lhsT=t2[:19, :128], rhs=wi2_t[:19, :],
                                 start=False, stop=True)
                inp_t = wp.tile([P, H], F32, tag="inp")
                nc.vector.tensor_copy(inp_t[:], pm[:])
                nc.sync.dma_start(out=inputs_d[r0:r1, :], in_=inp_t[:])
                m0_t = wp.tile([P, H], F16, tag="m0")
                nc.scalar.activation(m0_t[:], inp_t[:], mybir.ActivationFunctionType.Relu)
                nc.sync.dma_start(out=m_sh[0][r0:r1, :], in_=m0_t[:])
            nc.gpsimd.collective_compute(
                "AllGather", mybir.AluOpType.bypass, replica_groups=RG,
                ins=[m_sh[0][:]], outs=[m_full[0][:]])

            # ---------------- message-passing iterations ----------------
            for t in range(1, DEPTH):
                mf = m_full[(t + 1) % 2]
                mt = m_full[t % 2]
                msh = m_sh[t % 2]
                # atom phase: am = sum_j mf[a2b[a, j]]
                atom_phase_v3(mf)
                nc.gpsimd.collective_compute(
                    "AllGather", mybir.AluOpType.bypass, replica_groups=RG,
                    ins=[am_sh[:]], outs=[am_full[:]])
                # bond phase: m_t = relu(inputs + (am[b2a] - mf[rev]) @ W_h)
                for blk in range(nblkB):
                    r0, r1 = blk * P, (blk + 1) * P
                    gb = gp.tile([P, H], F16, tag="gb")
                    nc.gpsimd.indirect_dma_start(
                        out=gb[:], out_offset=None, in_=am_full[:],
                        in_offset=IndirectOffsetOnAxis(
                            ap=ixB_t[:, blk:blk + 1], axis=0))
                    gr = gp.tile([P, H], F16, tag="gr")
                    nc.gpsimd.indirect_dma_start(
                        out=gr[:], out_offset=None, in_=mf[:],
                        in_offset=IndirectOffsetOnAxis(
                            ap=ixR_t[:, blk:blk + 1], axis=0))
                    diff = wp.tile([P, H], F16, tag="diff")
                    nc.vector.tensor_sub(diff[:], gb[:], gr[:])
                    pdt = pp.tile([P, H], F16, tag="tp16")
                    nc.tensor.transpose(pdt[:], diff[:], id16[:])
                    dT = wp.tile([P, H], F16, tag="dT")
                    nc.vector.tensor_copy(dT[:], pdt[:])
                    pmm = pp2.tile([P, P], F32, tag="mm")
                    nc.tensor.matmul(pmm[:], lhsT=dT[:], rhs=wh_t[:], start=True, stop=True)
                    inp_t = wp.tile([P, H], F32, tag="inp")
                    nc.sync.dma_start(out=inp_t[:], in_=inputs_d[r0:r1, :])
                    pre = wp.tile([P, H], F32, tag="pre")
                    nc.vector.tensor_add(pre[:], pmm[:], inp_t[:])
                    mt_t = wp.tile([P, H], F16, tag="mt")
                    nc.scalar.activation(mt_t[:], pre[:], mybir.ActivationFunctionType.Relu)
                    nc.sync.dma_start(out=msh[r0:r1, :], in_=mt_t[:])
                nc.gpsimd.collective_compute(
                    "AllGather", mybir.AluOpType.bypass, replica_groups=RG,
                    ins=[msh[:]], outs=[mt[:]])

            # ---------------- final: atom_hiddens + per-molecule attention ----------------
            mf = m_full[(DEPTH - 1) % 2]
            atom_phase_v3(mf)
            for blk in range(nblkA):
                amf16 = wp.tile([P, H], F16, tag="amf16")
                nc.sync.dma_start(out=amf16[:],
                                  in_=am_sh[blk * P:(blk + 1) * P, :])
                amf = wp.tile([P, H], F32, tag="amf")
                nc.vector.tensor_copy(amf[:], amf16[:])
                # a_input = [f_atoms | 1 | am] @ W_o'  (b_o folded)
                fa_t = wp.tile([P, 134], F32, tag="fa")
                nc.sync.dma_start(out=fa_t[:], in_=fa[blk * P:(blk + 1) * P, :])
                pt1 = pp.tile([P, P], F32, tag="tp")
                nc.tensor.transpose(pt1[:], fa_t[:, 0:128], id32[:])
                tf1 = wp.tile([P, P], F32, tag="t1")
                nc.vector.tensor_copy(tf1[:], pt1[:])
                pt2 = pp.tile([P, P], F32, tag="tp")
                nc.tensor.transpose(pt2[:6, :], fa_t[:, 128:134], id32[:])
                tf2 = wp.tile([P, P], F32, tag="t2")
                nc.vector.tensor_copy(tf2[:6, :], pt2[:6, :])
                pt3 = pp.tile([P, P], F32, tag="tp")
                nc.tensor.transpose(pt3[:], amf[:], id32[:])
                tf3 = wp.tile([P, P], F32, tag="t3")
                nc.vector.tensor_copy(tf3[:], pt3[:])
                ph = pp2.tile([P, P], F32, tag="mm")
                nc.tensor.matmul(ph[:], lhsT=tf1[:], rhs=wo1_t[:], start=True, stop=False)
                nc.tensor.matmul(ph[:], lhsT=tf2[:6, :128], rhs=wo2_t[:6, :],
                                 start=False, stop=False)
                nc.tensor.matmul(ph[:], lhsT=tf3[:], rhs=wo3_t[:], start=False, stop=True)
                ah = wp.tile([P, H], F32, tag="ah")
                nc.scalar.activation(ah[:], ph[:], mybir.ActivationFunctionType.Relu)

                # ---- attention readout over MPB molecules in this block ----
                phT = pp.tile([P, P], F32, tag="tp")
                nc.tensor.transpose(phT[:], ah[:], id32[:])
                hT = wp.tile([P, P], F32, tag="hT")
                nc.vector.tensor_copy(hT[:], phT[:])
                pha = pp2.tile([P, P], F32, tag="mm")
                nc.tensor.matmul(pha[:], lhsT=wa_t[:], rhs=hT[:], start=True, stop=True)
                haT = wp.tile([P, P], F32, tag="haT")
                nc.vector.tensor_copy(haT[:], pha[:])
                psc = pp2.tile([P, P], F32, tag="mm")
                nc.tensor.matmul(psc[:], lhsT=haT[:], rhs=hT[:], start=True, stop=True)
                sc = wp.tile([P, P], F32, tag="sc")
                nc.vector.tensor_add(sc[:], psc[:], mask_t[:])
                mx = wp.tile([P, 1], F32, tag="mx")
                nc.vector.reduce_max(mx[:], sc[:], axis=mybir.AxisListType.X)
                e0 = wp.tile([P, P], F32, tag="e0")
                nc.vector.tensor_scalar_sub(e0[:], sc[:], mx[:])
                e = wp.tile([P, P], F32, tag="e")
                nc.scalar.activation(e[:], e0[:], mybir.ActivationFunctionType.Exp)
                sm = wp.tile([P, 1], F32, tag="sm")
                nc.vector.reduce_sum(sm[:], e[:], axis=mybir.AxisListType.X)
                rs = wp.tile([P, 1], F32, tag="rs")
                nc.vector.reciprocal(rs[:], sm[:])
                att = wp.tile([P, P], F32, tag="att")
                nc.vector.tensor_scalar_mul(att[:], e[:], rs[:])
                paT = pp.tile([P, P], F32, tag="tp")
                nc.tensor.transpose(paT[:], att[:], id32[:])
                attT = wp.tile([P, P], F32, tag="attT")
                nc.vector.tensor_copy(attT[:], paT[:])
                pz = pp2.tile([P, P], F32, tag="mm")
                nc.tensor.matmul(pz[:], lhsT=ah[:], rhs=attT[:], start=True, stop=True)
                zT = wp.tile([P, P], F32, tag="zT")
                nc.vector.tensor_copy(zT[:], pz[:])
                pah = pp2.tile([P, P], F32, tag="mm")
                nc.tensor.matmul(pah[:], lhsT=zT[:], rhs=wb_t[:], start=True, stop=True)
                rt = wp.tile([P, H], F32, tag="rt")
                nc.scalar.activation(rt[:], pah[:], mybir.ActivationFunctionType.Relu)
                tot = wp.tile([P, H], F32, tag="tot")
                nc.vector.tensor_add(tot[:], rt[:], ah[:])
                pmv = pp2.tile([MPB, H], F32, tag="pmv")
                nc.tensor.matmul(pmv[:], lhsT=g_t[:], rhs=tot[:], start=True, stop=True)
                mvo = sp.tile([P, H], F32, tag="mvs")
                nc.vector.tensor_copy(mvo[:MPB, :], pmv[:MPB, :])
                nc.sync.dma_start(out=mv[blk * MPB:(blk + 1) * MPB, :],
                                  in_=mvo[:MPB, :])
    nc.compile()
    return nc


def host_prep(f_atoms, f_bonds, W_i, W_h, W_o, b_o, W_a, W_b, b_b,
              a2b, b2a, b2revb, mol_size, A, B, AF, S):
    """Builds per-core in_maps."""
    As, Bs = A // NCORES, B // NCORES
    nblkA, nblkB = As // P, Bs // P
    MPB = P // S

    W_op = np.concatenate([W_o[:133], b_o[None, :], W_o[133:]], axis=0).astype(np.float32)
    fa_ext = np.concatenate([f_atoms, np.ones((A, 1), np.float32)], axis=1)
    amask = np.full((P, P), -30000.0, np.float32)
    for m in range(MPB):
        amask[m * S:(m + 1) * S, m * S:(m + 1) * S] = 0.0
    gsel = np.zeros((P, MPB), np.float32)
    for m in range(MPB):
        gsel[m * S:(m + 1) * S, m] = 1.0 / S

    common = dict(
        w_i=W_i.astype(np.float32), w_h=W_h.astype(np.float16),
        w_o=W_op, w_a=W_a.astype(np.float32), w_b=W_b.astype(np.float32),
        amask=amask, gsel=gsel,
    )
    SEGR_v = min(32768, B)
    NSEG = B // SEGR_v
    CAP_v = min(2560, As)
    CAPCOL_v = CAP_v // 16

    def pack_cells(cells):
        out = np.full((P, NB * NSEG * CAPCOL_v), -1, np.int16)
        for c in range(NB * NSEG):
            cell = cells[c].reshape(CAPCOL_v, 16).T  # [16, CAPCOL]
            out[:, c * CAPCOL_v:(c + 1) * CAPCOL_v] = np.tile(cell, (8, 1))
        return out

    in_maps = []
    for k in range(NCORES):
        a0, b0 = k * As, k * Bs
        a2b_s = a2b[a0:a0 + As]          # [As, NB]
        gcells = np.zeros((NB * NSEG, CAP_v), np.int16)
        scells = np.zeros((NB * NSEG, CAP_v), np.int16)
        for jg in range(NB):
            b = a2b_s[:, jg].astype(np.int64)
            seg = b // SEGR_v
            for s in range(NSEG):
                sel = np.nonzero(seg == s)[0]
                n = len(sel)
                assert n <= CAP_v, f"cell overflow {n} > {CAP_v}"
                c = jg * NSEG + s
                gcells[c, :n] = (b[sel] - s * SEGR_v).astype(np.int16)
                scells[c, :n] = sel.astype(np.int16)
        ixg_k = pack_cells(gcells)
        ixs_k = pack_cells(scells)
        idxA = np.ascontiguousarray(
            a2b_s.reshape(nblkA, P, NB).transpose(1, 0, 2).reshape(P, nblkA * NB)
        ).astype(np.int32)
        idxR = np.ascontiguousarray(
            b2revb[b0:b0 + Bs].reshape(nblkB, P).T).astype(np.int32)
        idxB = np.ascontiguousarray(
            b2a[b0:b0 + Bs].reshape(nblkB, P).T).astype(np.int32)
        in_maps.append(dict(
            fb=np.ascontiguousarray(f_bonds[b0:b0 + Bs]).astype(np.float32),
            fa=np.ascontiguousarray(fa_ext[a0:a0 + As]),
            idxA=idxA, idxR=idxR, idxB=idxB,
            ixg=ixg_k, ixs=ixs_k, **common))
    return in_maps


_NC_CACHE = {}


def get_nc(A, B, AF, S):
    key = (A, B, AF, S)
    if key not in _NC_CACHE:
        _NC_CACHE[key] = build_nc(A, B, AF, S)
    return _NC_CACHE[key]


def kernel(f_atoms, f_bonds, W_i, W_h, W_o, b_o, W_a, W_b, b_b,
           a2b, b2a, b2revb, mol_size):
    f_atoms = np.asarray(f_atoms, np.float32)
    f_bonds = np.asarray(f_bonds, np.float32)
    A, AF = f_atoms.shape
    B = f_bonds.shape[0]
    S = int(mol_size)
    nc = get_nc(A, B, AF, S)
    in_maps = host_prep(
        f_atoms, f_bonds, np.asarray(W_i), np.asarray(W_h), np.asarray(W_o),
        np.asarray(b_o), np.asarray(W_a), np.asarray(W_b), np.asarray(b_b),
        np.asarray(a2b), np.asarray(b2a), np.asarray(b2revb), S, A, B, AF, S)
    res = run_bass_kernel_spmd(nc, in_maps, core_ids=list(range(NCORES)))
    return np.concatenate([r["mv"] for r in res.results], axis=0)

